# revision 9
# baseline (speedup 1.0000x reference)
"""Causal multi-head self-attention on 8 TRN2 NeuronCores (Bass/Tile).

Problem (hardcoded): x[2, 2048, 1024], Wq/Wk/Wv/Wo [1024, 1024] (nn.Linear
convention, out x in), H=16 heads, dk=64, causal softmax, y = attn @ Wo.T.

Sharding: 2-way data parallel (batch) x 4-way tensor parallel (head groups of
4). Each core computes q/k/v projections for its 4 heads, causal attention,
and a partial output projection against its 256-column slice of Wo. The host
sums the 4 partial [2048, 1024] outputs per batch (the "all-reduce").

Device kernel design notes (v3):
  - All dense GEMMs (q/k/v projections and the Wo output projection) run as
    error-compensated fp8(e4m3) matmuls in DoubleRow perf mode: operands are
    split host-side (x, W) or on-device (attn out) into hi + lo fp8 parts and
    the three product terms (hi*hi, lo*hi, hi*lo) are accumulated, pairing
    k-tiles in the two DoubleRow slots. 2 k-tiles per matmul at 0.5
    cycles/row makes each GEMM 2.67x faster than fp16 while keeping relative
    error ~1e-3 (the dropped lo*lo term is ~0.1%).
  - Weights are pre-scaled by 32 host-side so their fp8-lo residuals stay in
    e4m3 normal range; the score scale (1/(32*32*sqrt(dk))) folds into the
    exp activation's scale argument, the v scale folds into the softmax
    denominator (ones column = 2 so out = 16*attn, putting the on-device
    outT hi/lo split in normal range too), and the final y comes out as
    512*y, divided on the host after the partial-sum reduce.
  - Attention itself (scores, exp, PV) stays fp16: raw fp8 noise (~3.6%)
    would blow the error budget and compensation there costs as much as
    fp16.
  - v is computed 4-heads-wide per j-tile ([128 j, 256] psum) instead of
    per-head: 4x fewer PE instructions for the same engine cycles.
  - Transposed score orientation (scoresT[j, i] per head) so exp'd
    probabilities land as [j-part, i-free] tiles directly usable by PV.
  - PV runs "flipped": p-tile [128j, 128i] is the stationary operand and the
    ones-augmented V [128j, 65] the moving one, so each accumulation matmul
    streams only 65 columns instead of 512 (PE time is proportional to the
    moving free dim). The softmax denominator lands per-PARTITION (row 64),
    where normalization is cheap: one strided reciprocal + one broadcast
    multiply on DVE -- no cross-partition broadcast matmul needed.
  - A PE transpose (with a staged identity) flips the normalized [i, dk-pair]
    block back to [dk-pair, i]; the fp16 transpose psum is then split into
    fp8 hi/lo on DVE for the Wo projection.
  - ScalarE's exp stream is the second-busiest engine, so score streams are
    emitted as early as dependencies allow (j-chunk by j-chunk behind the
    kT projections) and every score stream is interleaved with independent
    PE work (V j-tiles, qT chunks, Wo units) so the PE queue never
    stalls at its head waiting for an exp to drain a PSUM score tile.
  - Causal: tiles with j > i skipped; diagonal-crossing tiles compute only
    columns >= 128*r and a [128,128] triangular 0/1 mask is applied after
    exp. No max subtraction needed (|scores| <~ 7 pre-scale).
"""

import numpy as np
import ml_dtypes

import concourse.mybir as mybir
import concourse.tile as tile
from concourse import bacc
from concourse import bass_utils

F32 = mybir.dt.float32
FP16 = mybir.dt.float16
FP8 = mybir.dt.float8e4
DR = mybir.MatmulPerfMode.DoubleRow
EXP = mybir.ActivationFunctionType.Exp
MULT = mybir.AluOpType.mult
SUB = mybir.AluOpType.subtract
E4 = ml_dtypes.float8_e4m3

P = 128        # partitions
F = 512        # free-dim chunk (one PSUM bank of fp32)
D = 1024       # model dim
E = 256        # per-core head-group width (4 heads x 64)
DK = 64        # head dim
DK1 = DK + 1   # head dim + denominator column
HL = 4         # heads per core
NK = D // P    # contraction k-tiles for projections

W_SCALE = 32.0            # host pre-scale on all four weight matrices
EXP_SCALE = 1.0 / (W_SCALE * W_SCALE * 8.0)   # 1/(32*32*sqrt(64))
ONES_VAL = 2.0            # denominator column: out = (32/2)*attn = 16*attn
Y_SCALE = 512.0           # y' = 16*attn @ (32*Wo).T = 512*y

LAST_RESULTS = None  # test harness can inspect exec_time_ns etc.


def build(S: int = 2048):
    """Build the per-core Bass program (same program on all 8 cores)."""
    NIC = S // F     # i-chunks
    NJT = S // P     # j-tiles
    TPC = F // P     # j-tiles (and i-blocks) per i-chunk (4)

    nc = bacc.Bacc("TRN2", target_bir_lowering=False, debug=False,
                   enable_asserts=False)
    xh_d = nc.dram_tensor("xh", [D, S], FP8, kind="ExternalInput").ap()
    xl_d = nc.dram_tensor("xl", [D, S], FP8, kind="ExternalInput").ap()
    wq_d = nc.dram_tensor("wq", [D, 2, E], FP8, kind="ExternalInput").ap()
    wk_d = nc.dram_tensor("wk", [D, 2, E], FP8, kind="ExternalInput").ap()
    wv_d = nc.dram_tensor("wv", [D, 2, E], FP8, kind="ExternalInput").ap()
    wo_d = nc.dram_tensor("wo", [E, 2, D], FP8, kind="ExternalInput").ap()
    tri_d = nc.dram_tensor("tri", [P, P], FP16, kind="ExternalInput").ap()
    idn_d = nc.dram_tensor("idn", [P, P], FP16, kind="ExternalInput").ap()
    ones_d = nc.dram_tensor("ones", [P, 8], FP16, kind="ExternalInput").ap()
    y_d = nc.dram_tensor("y", [S, D], FP16, kind="ExternalOutput").ap()

    # long chunk first (feeds ScalarE early), short chunk last (small tail)
    IC_ORDER = list(range(NIC - 1, -1, -1))
    ic0 = IC_ORDER[0]

    with tile.TileContext(nc) as tc:
        with (
            tc.tile_pool(name="persist", bufs=1) as pp,
            tc.tile_pool(name="pT1", bufs=26) as ptp1,
            tc.tile_pool(name="pS1", bufs=17) as pts1,
            tc.tile_pool(name="pT2", bufs=15) as ptp2,
            tc.tile_pool(name="pS2", bufs=12) as pts2,
            tc.tile_pool(name="osb", bufs=4) as op,
            tc.tile_pool(name="rcp", bufs=4) as rcpp,
            tc.tile_pool(name="onsb", bufs=4) as onp,
            tc.tile_pool(name="ysb", bufs=5) as yp,
            tc.tile_pool(name="ps_s", bufs=2, space="PSUM") as pss,
            tc.tile_pool(name="ps_tw", bufs=3, space="PSUM") as pstw,
            tc.tile_pool(name="ps_tr", bufs=1, space="PSUM") as pstr,
        ):
            tri_sb = pp.tile([P, P], FP16)
            idn_sb = pp.tile([P, P], FP16)
            ones_sb = pp.tile([P, 8], FP16)
            qT_sb = pp.tile([P, E // P, S], FP16)
            kT_sb = pp.tile([P, E // P, S], FP16)
            v_sb = pp.tile([P, NJT, HL, DK1], FP16)
            wq_sb = pp.tile([P, NK, 2, E], FP8)
            wk_sb = pp.tile([P, NK, 2, E], FP8)
            wv_sb = pp.tile([P, NK, 2, E], FP8)
            xh_sb = pp.tile([P, NK, S], FP8)
            xl_sb = pp.tile([P, NK, S], FP8)
            wo_sb = pp.tile([P, E // P, 2, D], FP8)
            outT_sb = pp.tile([P, E // P, 2, S], FP8)

            xh_r = xh_d.rearrange("(kt p) s -> p kt s", p=P)
            xl_r = xl_d.rearrange("(kt p) s -> p kt s", p=P)
            wq_r = wq_d.rearrange("(kt p) two e -> p kt two e", p=P)
            wk_r = wk_d.rearrange("(kt p) two e -> p kt two e", p=P)
            wv_r = wv_d.rearrange("(kt p) two e -> p kt two e", p=P)
            wo_r = wo_d.rearrange("(et p) two o -> p et two o", p=P)

            # ---- input DMAs: ordered by the first-exp dependency chain ----
            # first exp needs qT[et0, chunk ic0] (wq + x[ic0]) and
            # kT[et0, j-chunk 0] (wk + x[jc0]); everything else after.
            # xh is needed by 2 of the 3 compensation terms, xl by the third,
            # so xh chunks always lead their xl partner.
            c0 = ic0 * F
            nc.sync.dma_start(wq_sb[:, :2], wq_r[:, :2])
            nc.sync.dma_start(xh_sb[:, :2, c0:c0 + F],
                              xh_r[:, :2, c0:c0 + F])
            nc.sync.dma_start(wq_sb[:, 2:], wq_r[:, 2:])
            nc.sync.dma_start(xh_sb[:, 2:, c0:c0 + F],
                              xh_r[:, 2:, c0:c0 + F])
            nc.sync.dma_start(xl_sb[:, :, c0:c0 + F], xl_r[:, :, c0:c0 + F])
            nc.sync.dma_start(wk_sb[:], wk_r[:])
            nc.sync.dma_start(tri_sb[:], tri_d)
            rest = [jc for jc in range(NIC) if jc != ic0]
            for ji, jc in enumerate(rest):
                nc.sync.dma_start(xh_sb[:, :, jc * F:(jc + 1) * F],
                                  xh_r[:, :, jc * F:(jc + 1) * F])
                nc.sync.dma_start(xl_sb[:, :, jc * F:(jc + 1) * F],
                                  xl_r[:, :, jc * F:(jc + 1) * F])
                if ji == 1:
                    nc.sync.dma_start(wv_sb[:], wv_r[:])
            if len(rest) < 2:
                nc.sync.dma_start(wv_sb[:], wv_r[:])
            nc.sync.dma_start(idn_sb[:], idn_d)
            nc.sync.dma_start(ones_sb[:], ones_d)
            nc.sync.dma_start(wo_sb[:], wo_r[:])

            # ones column of the augmented V (all j-tiles at once)
            nc.vector.tensor_copy(
                v_sb[:, :, :, DK].rearrange("p a b -> p (a b)"),
                ones_sb[:, :1].to_broadcast([P, NJT * HL]))

            # ---- emission helpers ----
            def s_mm(h, ic, jt, ps_out, col0):
                et = h // 2
                bp = (h % 2) * DK
                nc.tensor.matmul(
                    ps_out,
                    lhsT=kT_sb[bp:bp + DK, et, jt * P:(jt + 1) * P],
                    rhs=qT_sb[bp:bp + DK, et, ic * F + col0:(ic + 1) * F],
                    start=True, stop=True,
                )

            def s_group(h, ic, pools, jts, fill=None):
                """Scores + exp + causal mask for a subset of j-tiles of the
                (h, ic) stream. Full-width j-tile pairs share one 2-bank PSUM
                tile and one exp call; the four diagonal tiles pack their
                1280 trimmed columns into two 2-bank tiles (three exp calls
                -> two). fill() (optional) emits ~1us of independent PE work
                after each group so the PE queue is never headed by a matmul
                waiting on an exp drain. Returns [(pt_ap, off), ...] where
                j-tile slice for i-block b is pt_ap[:, off+b*P : off+(b+1)*P].
                """
                pair_pool, single_pool = pools
                ptiles = []
                jts = list(jts)
                i = 0
                while i < len(jts):
                    jt = jts[i]
                    r = jt - ic * TPC
                    if r < 0 and i + 1 < len(jts) and jts[i + 1] == jt + 1 \
                            and jts[i + 1] - ic * TPC < 0:
                        ps = pss.tile([P, 2 * F], F32, tag="pss", name="ps_s")
                        s_mm(h, ic, jt, ps[:, :F], 0)
                        s_mm(h, ic, jt + 1, ps[:, F:], 0)
                        pt = pair_pool.tile([P, 2 * F], FP16, tag="ptp",
                                            name="ptpair")
                        nc.scalar.activation(pt[:], ps[:], EXP,
                                             scale=EXP_SCALE)
                        ptiles.append((pt[:, :F], 0))
                        ptiles.append((pt[:, F:], 0))
                        i += 2
                    elif r == 0 and TPC == 4 and i + 3 < len(jts) \
                            and [jts[i + d] for d in range(4)] == \
                                [jt, jt + 1, jt + 2, jt + 3]:
                        # diagonal quad: r0|r1 and r2|r3 packed, 2 exps
                        ps1 = pss.tile([P, 2 * F], F32, tag="pss", name="psd1")
                        s_mm(h, ic, jt, ps1[:, 0:F], 0)          # r0 @ 0
                        s_mm(h, ic, jt + 1, ps1[:, F:F + 384], P)  # r1 @ 512
                        pt1 = pair_pool.tile([P, 2 * F], FP16, tag="ptp",
                                             name="ptd1")
                        nc.scalar.activation(pt1[:, :F + 384],
                                             ps1[:, :F + 384], EXP,
                                             scale=EXP_SCALE)
                        ps2 = pss.tile([P, 2 * F], F32, tag="pss", name="psd2")
                        s_mm(h, ic, jt + 2, ps2[:, 2 * P:F], 2 * P)  # r2 @ 256
                        s_mm(h, ic, jt + 3, ps2[:, F:F + P], 3 * P)  # r3 @ 512
                        pt2 = pair_pool.tile([P, 2 * F], FP16, tag="ptp",
                                             name="ptd2")
                        nc.scalar.activation(pt2[:, 2 * P:F + P],
                                             ps2[:, 2 * P:F + P], EXP,
                                             scale=EXP_SCALE)
                        for pt, o in ((pt1, 0), (pt1, F), (pt2, 2 * P),
                                      (pt2, F)):
                            nc.gpsimd.tensor_tensor(
                                pt[:, o:o + P], pt[:, o:o + P],
                                tri_sb[:], MULT)
                        ptiles += [(pt1, 0), (pt1, 384), (pt2, 0), (pt2, P)]
                        i += 4
                        if fill is not None:
                            fill()
                    else:
                        col0 = max(0, r * P)
                        ps = pss.tile([P, 2 * F], F32, tag="pss", name="ps_s")
                        s_mm(h, ic, jt, ps[:, col0:F], col0)
                        pt = single_pool.tile([P, F], FP16, tag="pts",
                                              name="ptsing")
                        nc.scalar.activation(pt[:, col0:], ps[:, col0:F], EXP,
                                             scale=EXP_SCALE)
                        if r >= 0:
                            nc.gpsimd.tensor_tensor(
                                pt[:, col0:col0 + P], pt[:, col0:col0 + P],
                                tri_sb[:], MULT)
                        ptiles.append((pt, 0))
                        i += 1
                    if fill is not None:
                        fill()
                return ptiles

            def proj_group(w, dst_sb, et, ic):
                """Compensated fp8 DR projection: dst[:, et, ic*F:] =
                (W_hi+W_lo).T @ (xh+xl) less the lo*lo term. k-tiles pair up
                in the DoubleRow slots; terms needing only xh go first so the
                group can start before xl's DMA lands."""
                pt = pstw.tile([P, F], F32, tag="pstw", name="psP")
                e0 = et * P
                c = ic * F
                for t in range(0, NK, 2):
                    nc.tensor.matmul(
                        pt,
                        lhsT=w[:, t:t + 2, 0, e0:e0 + P],
                        rhs=xh_sb[:, t:t + 2, c:c + F],
                        start=(t == 0), stop=False, perf_mode=DR,
                    )
                    nc.tensor.matmul(
                        pt,
                        lhsT=w[:, t:t + 2, 1, e0:e0 + P],
                        rhs=xh_sb[:, t:t + 2, c:c + F],
                        start=False, stop=False, perf_mode=DR,
                    )
                for t in range(0, NK, 2):
                    nc.tensor.matmul(
                        pt,
                        lhsT=w[:, t:t + 2, 0, e0:e0 + P],
                        rhs=xl_sb[:, t:t + 2, c:c + F],
                        start=False, stop=(t == NK - 2), perf_mode=DR,
                    )
                nc.vector.tensor_copy(dst_sb[:, et, c:c + F], pt)

            def v_group(jt):
                """All 4 heads' V rows for one j-tile: v[jt, :] =
                x[jt].T @ (Wv_hi+Wv_lo), compensated fp8 DR with x as the
                stationary side."""
                vt = pstw.tile([P, F], F32, tag="pstw", name="psV")[:, :E]
                j0 = jt * P
                for t in range(0, NK, 2):
                    nc.tensor.matmul(
                        vt,
                        lhsT=xh_sb[:, t:t + 2, j0:j0 + P],
                        rhs=wv_sb[:, t:t + 2, 0, :],
                        start=(t == 0), stop=False, perf_mode=DR,
                    )
                    nc.tensor.matmul(
                        vt,
                        lhsT=xh_sb[:, t:t + 2, j0:j0 + P],
                        rhs=wv_sb[:, t:t + 2, 1, :],
                        start=False, stop=False, perf_mode=DR,
                    )
                    nc.tensor.matmul(
                        vt,
                        lhsT=xl_sb[:, t:t + 2, j0:j0 + P],
                        rhs=wv_sb[:, t:t + 2, 0, :],
                        start=False, stop=(t == NK - 2), perf_mode=DR,
                    )
                nc.vector.tensor_copy(
                    v_sb[:, jt, :, :DK],
                    vt.rearrange("p (h d) -> p h d", d=DK))

            def emit_wo_unit(ic, u, tail=False, late=False):
                it = ic * TPC + u
                i0 = it * P
                yt = yp.tile([P, D], FP16, tag="y", name="yt")
                for oc in range(D // F):
                    if (tail or late) and oc == 1:
                        # the score PSUM pool is idle by the tail: borrow it
                        # as two extra ring slots for the Wo pipeline
                        ps = pss.tile([P, 2 * F], F32, tag="pss",
                                      name="ps_yb")[:, :F]
                    else:
                        ps = pstw.tile([P, F], F32, tag="pstw", name="ps_y")
                    o0 = oc * F
                    nc.tensor.matmul(
                        ps,
                        lhsT=outT_sb[:, :, 0, i0:i0 + P],
                        rhs=wo_sb[:, :, 0, o0:o0 + F],
                        start=True, stop=False, perf_mode=DR,
                    )
                    nc.tensor.matmul(
                        ps,
                        lhsT=outT_sb[:, :, 0, i0:i0 + P],
                        rhs=wo_sb[:, :, 1, o0:o0 + F],
                        start=False, stop=False, perf_mode=DR,
                    )
                    nc.tensor.matmul(
                        ps,
                        lhsT=outT_sb[:, :, 1, i0:i0 + P],
                        rhs=wo_sb[:, :, 0, o0:o0 + F],
                        start=False, stop=True, perf_mode=DR,
                    )
                    # copies rotate across engines in the last stretch so
                    # the PSUM ring frees at matmul pace (exp stream is done
                    # there, so ScalarE/DVE have slack)
                    dst = yt[:, o0:o0 + F]
                    if tail or late:
                        if getattr(emit_wo_unit, "rot", 1) == 0:
                            nc.scalar.copy(dst, ps)
                            emit_wo_unit.rot = 1
                        else:
                            nc.vector.tensor_copy(dst, ps)
                            emit_wo_unit.rot = 0
                    else:
                        nc.vector.tensor_copy(dst, ps)
                nc.sync.dma_start(y_d[i0:i0 + P, :], yt[:])

            def pv_flip(h, ic, ptiles):
                """Flipped PV: out[i-block, dk+1] per 128-i-block, p as
                stationary, augmented V as 65-wide moving."""
                ps_o = pstw.tile([P, F], F32, tag="pstw", name="ps_o")[:, :TPC * DK1]
                # b-outer so accumulation groups are sequential in the bank
                for b in range(TPC):
                    for jt in range(ic * TPC + b + 1):
                        pt, off = ptiles[jt]
                        nc.tensor.matmul(
                            ps_o[:, b * DK1:(b + 1) * DK1],
                            lhsT=pt[:, off + b * P:off + (b + 1) * P],
                            rhs=v_sb[:, jt, h, :],
                            start=(jt == 0), stop=(jt == ic * TPC + b),
                        )
                return ps_o

            def copy_out(ps_o, tail=False):
                """Drain the PV psum to SBUF (frees the bank for the next
                head's PV as soon as possible). In the tail the exp stream is
                done, so route through the idle scalar engine instead of the
                congested DVE queue."""
                o_t = op.tile([P, TPC, DK1], F32, tag="o", name="o_t")
                if tail:
                    nc.scalar.copy(o_t.rearrange("p a b -> p (a b)"), ps_o)
                else:
                    nc.vector.tensor_copy(
                        o_t.rearrange("p a b -> p (a b)"), ps_o)
                return o_t

            def norm(h, o_t, on_t):
                """Reciprocal the per-partition denominators, apply to the
                dk columns -> on_t half."""
                rc = rcpp.tile([P, TPC], F32, tag="rcp", name="rc")
                nc.vector.reciprocal_approx_fast(
                    out=rc[:], in_=o_t[:, :, DK])
                hh = h % 2
                nc.vector.tensor_tensor(
                    on_t[:, :, hh * DK:(hh + 1) * DK],
                    o_t[:, :, :DK],
                    rc[:].to_broadcast([P, TPC, DK]), MULT)

            def transp(et, ic, on_t, tail=False):
                """Transpose normalized [i, dk-pair] blocks back to
                [dk-pair, i], then split the fp16 psum into fp8 hi/lo for
                the compensated Wo projection."""
                ps_t = pstr.tile([P, F], FP16, tag="pstr", name="ps_t")
                for b in range(TPC):
                    nc.tensor.matmul(
                        ps_t[:, b * P:(b + 1) * P],
                        lhsT=on_t[:, b, :],
                        rhs=idn_sb[:],
                        start=True, stop=True, is_transpose=True,
                    )
                c = ic * F
                hi = outT_sb[:, et, 0, c:c + F]
                if tail:
                    nc.scalar.copy(hi, ps_t)
                else:
                    nc.vector.tensor_copy(hi, ps_t)
                nc.vector.tensor_tensor(
                    outT_sb[:, et, 1, c:c + F], ps_t, hi, SUB)

            # ---- fillers: independent PE work threaded into score streams
            fillers = []  # (head_tag_or_None, closure)

            def fill():
                if fillers:
                    fillers.pop(0)[1]()

            def drain_v(h):
                """Force-emit any V fillers head h's PV still needs."""
                rest = []
                for tag, fn in fillers:
                    if tag == h:
                        fn()
                    else:
                        rest.append((tag, fn))
                fillers[:] = rest

            def s_stream(h, ic, pools, jcs=None):
                """Emit the full (h, ic) stream, optionally j-chunk by
                j-chunk in a custom order; returns {jt: (pt, off)}."""
                if jcs is None:
                    jts = list(range((ic + 1) * TPC))
                    return dict(zip(jts, s_group(h, ic, pools, jts, fill)))
                out = {}
                for jc in jcs:
                    jts = [t for t in range((ic + 1) * TPC)
                           if t // TPC == jc]
                    out.update(zip(jts, s_group(h, ic, pools, jts, fill)))
                return out

            # ---- Phase A: et0 projections j-chunk-paced under the first
            # score stream, then the remaining first-chunk streams. Stream 0
            # opens with its diagonal j-chunk: that only needs the first
            # chunk's x slice, which is the first DMA to land.
            proj_group(wq_sb, qT_sb, 0, ic0)
            streams = {}
            cur = {}
            # et1 projections self-pace as fillers inside stream 0: each is
            # popped right after the jc group whose x slice it also needs.
            proj_group(wq_sb, qT_sb, 1, ic0)
            fillers += [(None, lambda jc=jc: proj_group(wk_sb, kT_sb, 1, jc))
                        for jc in range(max(0, NIC - 2))]
            if NIC > 1:  # late s0 groups can hide the first V j-tiles
                fillers += [(0, lambda jt=jt: v_group(jt))
                            for jt in range(4)]
            jc_order = [ic0] + [jc for jc in range(NIC) if jc != ic0]
            for jc in jc_order:
                proj_group(wk_sb, kT_sb, 0, jc)
                jts = [t for t in range((ic0 + 1) * TPC) if t // TPC == jc]
                cur.update(zip(jts, s_group(0, ic0, (ptp1, pts1), jts, fill)))
            streams[0] = cur
            fillers += [(None, lambda jc=jc: proj_group(wk_sb, kT_sb, 1, jc))
                        for jc in range(max(0, NIC - 2), NIC)]
            fillers += [(0, lambda jt=jt: v_group(jt))
                        for jt in range(4 if NIC > 1 else 0, NJT // 3)]
            streams[1] = s_stream(1, ic0, (ptp1, pts1))
            while fillers and fillers[0][0] is None:
                fill()
            fillers += [(0, lambda jt=jt: v_group(jt))
                        for jt in range(NJT // 3, 2 * NJT // 3)]
            streams[2] = s_stream(2, ic0, (ptp1, pts1))
            fillers += [(0, lambda jt=jt: v_group(jt))
                        for jt in range(2 * NJT // 3, NJT)]
            streams[3] = s_stream(3, ic0, (ptp1, pts1))

            # ---- Phase B: per-chunk PV/normalize/transpose/Wo with the
            # next chunk's four score streams threaded through. The previous
            # chunk's Wo units ride the filler queue so they land inside the
            # score streams exactly where the PE would otherwise stall
            # waiting for an exp to free a score-PSUM slot. ----
            prev_ic = None
            for idx_ic, ic in enumerate(IC_ORDER):
                in_tail = idx_ic == len(IC_ORDER) - 1
                nic = (IC_ORDER[idx_ic + 1]
                       if idx_ic + 1 < len(IC_ORDER) else None)
                pts = streams
                nstreams = {}

                on_t = {e: onp.tile([P, TPC, P], FP16, tag="on",
                                    name=f"on_{e}") for e in range(2)}

                # in the tail there are no next-chunk streams to fill, so
                # front-load the prev chunk's units directly
                wo_sched = {0: [0, 1], 1: [2], 2: [3], 3: []}

                def wo_step(s):
                    if prev_ic is not None and in_tail:
                        for u in wo_sched[s]:
                            emit_wo_unit(prev_ic, u, tail=True, late=True)

                # V j-tiles were emitted as fillers of the first chunk's
                # streams; all of them must land before the first PV.
                drain_v(0)
                if prev_ic is not None and not in_tail:
                    while fillers:
                        fill()
                    fillers.extend(
                        (None, lambda u=u: emit_wo_unit(prev_ic, u))
                        for u in range(TPC))
                pso0 = pv_flip(0, ic, pts[0])
                ot0 = copy_out(pso0, tail=in_tail)
                wo_step(0)
                if nic is not None:
                    proj_group(wq_sb, qT_sb, 0, nic)
                    nstreams[0] = s_stream(0, nic, (ptp1, pts1))
                norm(0, ot0, on_t[0])
                pso1 = pv_flip(1, ic, pts[1])
                ot1 = copy_out(pso1, tail=in_tail)
                wo_step(1)
                if nic is not None:
                    nstreams[1] = s_stream(1, nic, (ptp2, pts2))
                norm(1, ot1, on_t[0])
                transp(0, ic, on_t[0], tail=in_tail)
                pso2 = pv_flip(2, ic, pts[2])
                ot2 = copy_out(pso2, tail=in_tail)
                wo_step(2)
                if nic is not None:
                    proj_group(wq_sb, qT_sb, 1, nic)
                    nstreams[2] = s_stream(2, nic, (ptp1, pts1))
                norm(2, ot2, on_t[1])
                pso3 = pv_flip(3, ic, pts[3])
                ot3 = copy_out(pso3, tail=in_tail)
                wo_step(3)
                if nic is not None:
                    nstreams[3] = s_stream(3, nic, (ptp2, pts2))
                norm(3, ot3, on_t[1])
                transp(1, ic, on_t[1], tail=in_tail)
                streams = nstreams
                prev_ic = ic
            while fillers:
                fill()
            for u in range(TPC):
                emit_wo_unit(prev_ic, u, tail=True)

    nc.compile()
    return nc


_CACHE = {}


def _get_nc(S):
    if S not in _CACHE:
        _CACHE[S] = build(S)
    return _CACHE[S]


def _split8(a):
    """fp32 array -> (hi, lo) e4m3 pair with hi+lo ~= a."""
    hi = a.astype(E4)
    lo = (a - hi.astype(np.float32)).astype(E4)
    return hi, lo


def kernel(x, Wq, Wk, Wv, Wo):
    global LAST_RESULTS
    x = np.asarray(x, dtype=np.float32)
    Wq = np.asarray(Wq, dtype=np.float32)
    Wk = np.asarray(Wk, dtype=np.float32)
    Wv = np.asarray(Wv, dtype=np.float32)
    Wo = np.asarray(Wo, dtype=np.float32)
    B, S, D_ = x.shape
    nc = _get_nc(S)

    tri = np.triu(np.ones((P, P), np.float16))          # keep j' <= t
    idn = np.eye(P, dtype=np.float16)
    ones = np.full((P, 8), ONES_VAL, np.float16)

    def pack_w(Wslice):
        """[E, D] nn.Linear slice -> [D, 2, E] scaled fp8 hi/lo."""
        wt = np.ascontiguousarray(Wslice.T) * np.float32(W_SCALE)
        hi, lo = _split8(wt)
        out = np.empty((D_, 2, Wslice.shape[0]), E4)
        out[:, 0, :] = hi
        out[:, 1, :] = lo
        return out

    in_maps = []
    for c in range(8):
        b, g = divmod(c, 4)
        sl = slice(E * g, E * (g + 1))
        xT = np.ascontiguousarray(x[b].T)
        xh, xl = _split8(xT)
        wot = np.ascontiguousarray(Wo[:, sl].T) * np.float32(W_SCALE)
        woh, wol = _split8(wot)
        wo_hl = np.empty((E, 2, D_), E4)
        wo_hl[:, 0, :] = woh
        wo_hl[:, 1, :] = wol
        in_maps.append({
            "xh": xh,
            "xl": xl,
            "wq": pack_w(Wq[sl]),
            "wk": pack_w(Wk[sl]),
            "wv": pack_w(Wv[sl]),
            "wo": wo_hl,
            "tri": tri,
            "idn": idn,
            "ones": ones,
        })

    res = bass_utils.run_bass_kernel_spmd(nc, in_maps, core_ids=list(range(8)))
    LAST_RESULTS = res

    y = np.zeros((B, S, D_), np.float32)
    for c in range(8):
        y[c // 4] += res.results[c]["y"].astype(np.float32)
    y *= np.float32(1.0 / Y_SCALE)
    return y


if __name__ == "__main__":
    # small-S self test against numpy
    S = 512
    rng = np.random.default_rng(0)
    B, H, dk = 2, 16, 64
    x = rng.standard_normal((B, S, D)).astype(np.float32)
    sc = 1.0 / np.sqrt(D)
    Wq = (rng.standard_normal((D, D)) * sc).astype(np.float32)
    Wk = (rng.standard_normal((D, D)) * sc).astype(np.float32)
    Wv = (rng.standard_normal((D, D)) * sc).astype(np.float32)
    Wo = (rng.standard_normal((D, D)) * sc).astype(np.float32)

    def ref(x, Wq, Wk, Wv, Wo):
        x64 = x.astype(np.float64)
        q = (x64 @ Wq.T.astype(np.float64)).reshape(B, S, H, dk).transpose(0, 2, 1, 3)
        k = (x64 @ Wk.T.astype(np.float64)).reshape(B, S, H, dk).transpose(0, 2, 1, 3)
        v = (x64 @ Wv.T.astype(np.float64)).reshape(B, S, H, dk).transpose(0, 2, 1, 3)
        s = np.einsum("bhid,bhjd->bhij", q, k) / np.sqrt(dk)
        mask = np.triu(np.ones((S, S), bool), k=1)
        s = np.where(mask, -np.inf, s)
        s -= s.max(axis=-1, keepdims=True)
        p = np.exp(s)
        p /= p.sum(axis=-1, keepdims=True)
        o = np.einsum("bhij,bhjd->bhid", p, v).transpose(0, 2, 1, 3).reshape(B, S, D)
        return o @ Wo.T.astype(np.float64)

    expected = ref(x, Wq, Wk, Wv, Wo)
    actual = kernel(x, Wq, Wk, Wv, Wo)
    err = np.abs(actual - expected).max() / np.abs(expected).max()
    print("self-test S=512 max rel err:", err)
    assert err < 1e-2, err
    print("PASS")
